# revision 67
# baseline (speedup 1.0000x reference)
"""Trainium2 Bass kernel for nn_AttentionAugmentation.

Attention with 2D relative-position logits. B=8, H=W=32, dk=dv=256, Nh=8.
Sharding: data-parallel over batch (one batch per NeuronCore, 8 cores).

Per-core v4 (one batch, 8 heads of 1024x1024 attention, dkh=32):
  - fp16 datapath end-to-end (inputs cast fp32->fp16 in the SWDGE DMA):
    q/k/v, qaug/kaug, rel keys, wexp are all fp16 -- higher precision than
    bf16 buys the error budget for the cheap DVE exp below.
  - PE warm-up: a dense stream of N=512 matmuls opens the HAM clock gate
    (1.2 -> 2.4 GHz) under the input DMA (transposes don't count for HAM).
  - qT/kT via PE transposes -> cast -> partition-scatter DMAs into
    qaug/kaug rows 0-31; q^T is also replicated to qaug rows 96-127.
    dk^-0.5 is folded into the exp (ACT scale / Schraudolph A).
  - rel logits in a 96-row augmented contraction: kaug rows 32-63 one-hot
    of key y2, rows 64-95 one-hot of key x2; qaug rows 32-63 = WRELT,
    rows 64-95 = HRELT, computed by shifted krw/krh^T-window matmuls.
    These are K=32 contractions, so they are emitted in psum-tile PAIRS
    with the second tile's matmuls on PE row-group 96 (q/krwT/krhT
    replicas at partitions 96-127) -> the two streams run concurrently.
    H psum is copied straight to qaug (runs of 32, ACT/DVE); W psum is
    staged contiguously (DVE/ACT) and scattered (runs of 1) by GPSIMD
    cores 2-3, which own partitions 32-63 -- a runs-of-1 strided copy
    costs ~4.7us on DVE/ACT vs ~2.2us on the otherwise-idle GPSIMD.
  - attention per head-pair, software-pipelined per 128-key chunk:
    S^T = QK matmuls (f32 psum); per slot ONE head's exp runs on ACT
    (activation Exp -> fp16) and the other on DVE as a single-op fp16
    Schraudolph: tensor_scalar i16(A*logit + B) whose int16 bits ARE the
    fp16 exponent+mantissa staircase (~1.8% rms, C=56 kills the mean bias
    so mixing with ACT chunks inside one softmax row is safe; end-to-end
    rel-err 1.4e-2 < 2e-2). The i16 tile is bitcast to fp16 as the AV rhs.
    AV uses lhsT=[V | 1] per head; the two heads of a pair write att psum
    partitions 0-32 / 64-96 (col-tiled concurrent matmuls).
  - output per pair, pipelined per 512-col half: att psum -> fp16 SBUF
    (x2^-6; ACT half / DVE half in parallel), xbar DMA-transpose per half,
    reciprocal per half, then per-(head,chunk) reciprocal-scaled copies
    into out_sb (hh=0 on DVE / hh=1 on ACT; ~10/6 DVE/ACT on the last,
    tail-serial pair), and a per-pair DMA of the 64 output channels (the
    last pair's goes out in two pixel-half DMAs, and its second transpose
    issues on the by-then-idle scalar queue so the issues overlap).
"""
import sys

sys.path.insert(0, "/opt/trn_rl_repo")

from contextlib import ExitStack

import numpy as np

import concourse.bass as bass
from concourse import bacc
import concourse.mybir as mybir
from concourse import masks
from concourse.tile import TileContext

HW = 1024
CH = 768
NH = 8
F32 = mybir.dt.float32
FP16 = mybir.dt.float16
I16 = mybir.dt.int16
EXP = mybir.ActivationFunctionType.Exp
COPY = mybir.ActivationFunctionType.Copy
MULT = mybir.AluOpType.mult
ADD = mybir.AluOpType.add
QSCALE = float((256 / 8) ** -0.5)
# fp16 single Schraudolph: i1 = int16(A*x + B) bitcast fp16 ~= exp(x*QSCALE)
# in ONE DVE op (~1.2us/chunk vs ACT's 1.13): the int16 staircase IS the
# fp16 exponent+mantissa. A folds QSCALE; C=56 zeroes the mean bias so
# ACT-exp and DVE-exp chunks mix inside one softmax row (per-weight rms
# ~1.8% averages out in the AV sum; end-to-end sim: 1.3e-2 < 2e-2).
SCH_A = 1024.0 * QSCALE / np.log(2.0)
SCH_B1 = 15360.0 - 56.0
# (hh, c) chunks computed on the DVE path, per pair index: one per slot so
# no slot serializes two ACT exps (except pair 0, whose early slots stay on
# ACT because the DVE is still busy with phase-A work).
DVE_CHUNKS = {
    0: {(0, 4), (0, 6), (1, 5), (1, 7)},
    1: {(0, c) for c in range(0, 8, 2)} | {(1, c) for c in range(1, 8, 2)},
    2: {(0, c) for c in range(0, 8, 2)} | {(1, c) for c in range(1, 8, 2)},
    3: {(0, c) for c in range(0, 8, 2)} | {(1, c) for c in range(1, 8, 2)},
}


def build_nc():
    nc = bacc.Bacc()
    # input split in two halves: a single [1024, 768] parameter makes the
    # axon-pjrt reshard program's dynamic-slice exceed a 16-bit semaphore
    # field in neuronx-cc (25MB concat across 8 cores), crashing walrus.
    xa_d = nc.declare_dram_parameter("xa", [HW // 2, CH], F32, isOutput=False)
    xb_d = nc.declare_dram_parameter("xb", [HW // 2, CH], F32, isOutput=False)
    krw_d = nc.declare_dram_parameter("krw", [63, 32], F32, isOutput=False)
    krh_d = nc.declare_dram_parameter("krh", [63, 32], F32, isOutput=False)
    out_d = nc.declare_dram_parameter("out", [HW, 256], F32, isOutput=True)

    with ExitStack() as octx:
        tc = octx.enter_context(TileContext(nc))
        sb = octx.enter_context(tc.tile_pool(name="persist", bufs=1))

        x_sb = sb.tile([128, 8 * CH], FP16)     # natural input: part p, col 768c+ch
        # qaug rows 96-127 hold a REPLICA of rows 0-31 (q^T): the rel-logit
        # matmuls are K=32 contractions, so running them pairwise in PE row
        # groups 0 and 96 doubles their throughput (the [96,*] tile already
        # reserves the column range on all 128 partitions -- replica is free).
        qaug = sb.tile([128, NH * HW], FP16)    # per head h: cols 1024h + (32x + y)
        kaug = sb.tile([96, NH * HW], FP16)
        v1 = sb.tile([128, NH * 8 * 33], FP16)  # per (h,c): 33 cols = V chunk | ones
        tscr = sb.tile([128, 2 * HW], FP16)     # transpose scratch (2 groups live)
        wnat = sb.tile([64, NH * HW], FP16)     # rows 32-63: W rel, (y, h, x) major
        out_sb = sb.tile([128, 8 * 256], F32)   # col 256c + ch
        identb = sb.tile([128, 128], FP16)
        krw_sb = sb.tile([63, 32], FP16)
        krh_sb = sb.tile([63, 32], FP16)
        # zero-padded transposed rel keys: walrus rejects 32-contraction
        # matmuls whose psum out starts at partition 32/64, so the rel MMs
        # use wider lhsT windows that land the useful rows at 32-63 (W) /
        # 64-95 (H) of a base-0 psum tile instead.
        krwT = sb.tile([128, 128], FP16)  # krwT[:, 32+m] = krw^T[:, m]
        krhT = sb.tile([128, 160], FP16)  # krhT[:, 64+m] = krh^T[:, m]
        # (rows 96-127 of krwT/krhT replicate rows 0-31 for row-group 96)

        # ---- constants first: identity lands fast so the PE warm-up can
        # start while the input DMAs stream ----
        masks.make_identity(nc, identb[:])

        # ---- input DMAs (SWDGE: fp32 -> fp16 cast). krw/krh go first (tiny
        # transfers; the SWDGE queue is FIFO and anything after 3MB of x
        # would land ~20us in). x loads q cols first. ----
        # V1 "ones" memset first: the PE warm-up below streams v1 as its rhs
        # (contents irrelevant), so it must be written early.
        v1v = v1[:].rearrange("p (h c e) -> p h c e", h=8, c=8, e=33)
        nc.gpsimd.memset(v1v[:, :, :, 32], 1.0)
        nc.gpsimd.dma_start(out=krw_sb[:], in_=krw_d[:])
        nc.gpsimd.dma_start(out=krh_sb[:], in_=krh_d[:])
        xv = x_sb[:].rearrange("p (c g) -> p c g", c=8, g=768)
        for col0 in (0, 256, 512):              # q, k, v column groups
            for half, src_d in ((0, xa_d), (1, xb_d)):
                nc.gpsimd.dma_start(
                    out=xv[:, 4 * half:4 * half + 4, col0:col0 + 256],
                    in_=src_d[:].rearrange("(c p) g -> p c g", p=128)
                        [:, :, col0:col0 + 256],
                )
        # one-hot rows of kaug, head-0 block only: rows 32-63: [y2(k)==j],
        # rows 64-95: [x2(k)==j]; col = 32*x2 + y2. Then DMA-replicate to
        # the other 7 head blocks (log-doubling).
        nc.gpsimd.memset(kaug[32:64, 0:HW], 0.0)
        nc.gpsimd.memset(kaug[64:96, 0:HW], 0.0)
        nc.gpsimd.affine_select(
            out=kaug[32:64, 0:HW].rearrange("p (x y) -> p x y", x=32, y=32),
            in_=kaug[32:64, 0:HW].rearrange("p (x y) -> p x y", x=32, y=32),
            compare_op=mybir.AluOpType.not_equal,
            fill=1.0,
            base=0,
            pattern=[[0, 32], [-1, 32]],
            channel_multiplier=1,
        )
        nc.gpsimd.affine_select(
            out=kaug[64:96, 0:HW].rearrange("p (x y) -> p x y", x=32, y=32),
            in_=kaug[64:96, 0:HW].rearrange("p (x y) -> p x y", x=32, y=32),
            compare_op=mybir.AluOpType.not_equal,
            fill=1.0,
            base=0,
            pattern=[[-1, 32], [0, 32]],
            channel_multiplier=1,
        )
        n = HW
        while n < NH * HW:
            rep = min(n, NH * HW - n)
            nc.sync.dma_start(
                out=kaug[32:96, n:n + rep],
                in_=kaug[32:96, 0:rep],
            )
            n += rep
        # V1: ones in col 32 of each 33-block (memset above); V chunks fill
        # cols 0-31 via copies emitted below on GPSIMD.
        v1c = v1[:].rearrange("p (h c e) -> p c h e", h=8, c=8, e=33)

        # ================= Phase A: transposes + rel logits =================
        with ExitStack() as actx:
            psA = actx.enter_context(tc.tile_pool(name="psA", bufs=2, space="PSUM"))
            psR = actx.enter_context(tc.tile_pool(name="psR", bufs=3, space="PSUM"))

            # PE warm-up: a DENSE stream of N=512 matmuls so the HAM clock
            # gate opens (1.2 -> 2.4 GHz) while the input DMA streams in.
            # Transpose-mode does not count as PE-busy for HAM, and short
            # N=128 matmuls (70%-duty) were measured not to open it either.
            # rhs = v1 (contents irrelevant; its ones-memset lands first).
            wps = psA.tile([128, HW], FP16, tag="tps")
            wpsf = wps[:].bitcast(F32)
            # 12 MMs (~7.6us cold): trimming to 8 measured +17us -- the
            # stream must bridge the gap until the qT input data arrives,
            # or the gate re-closes and all of phase A runs at 1.2 GHz.
            for i in range(12):
                nc.tensor.matmul(
                    out=wpsf[:, 0:512], lhsT=identb[:], rhs=v1[:, 0:512],
                    start=True, stop=True,
                )
            nc.vector.memset(krwT[:], 0.0)
            nc.vector.memset(krhT[:], 0.0)

            # qT/kT: PE-transpose 4-head groups, cast, partition-scatter
            # DMAs into qaug/kaug rows 0-31. (Tried [128,128] XBAR DMA
            # transposes instead: +45us -- per-op xbar issue cost dominates
            # at this size. PE transposes win.)
            def transpose_group(kind, g, dstt):
                col0 = 256 * kind + 128 * g
                pt = psA.tile([128, HW], FP16, tag="tps")
                for c in range(8):
                    nc.tensor.transpose(
                        out=pt[:, 128 * c:128 * c + 128],
                        in_=x_sb[:, 768 * c + col0:768 * c + col0 + 128],
                        identity=identb[:, 0:128],
                    )
                scr = tscr[:, HW * g:HW * g + HW]
                # casts split across DVE/ACT so the two groups overlap
                if g == 0:
                    nc.vector.tensor_copy(out=scr, in_=pt[:])
                else:
                    nc.scalar.copy(out=scr, in_=pt[:])
                for hh in range(4):
                    h = 4 * g + hh
                    dma_eng = nc.sync if hh % 2 == 0 else nc.scalar
                    dma_eng.dma_start(
                        out=dstt[0:32, HW * h:HW * h + HW],
                        in_=tscr[32 * hh:32 * hh + 32, HW * g:HW * g + HW],
                    )
                    if kind == 0:
                        # q^T replica at partitions 96-127 (row-group 96
                        # operand for the packed rel matmuls)
                        dma_eng.dma_start(
                            out=dstt[96:128, HW * h:HW * h + HW],
                            in_=tscr[32 * hh:32 * hh + 32, HW * g:HW * g + HW],
                        )

            for g in range(2):
                transpose_group(0, g, qaug)

            # key_rel transposes: krw [63,32] -> krwT cols 32-94 (zero pad);
            # then DMA-replicate rows 0-31 into rows 96-127 (row-group 96).
            for srct, dst, off in ((krw_sb, krwT, 32), (krh_sb, krhT, 64)):
                pt = psA.tile([128, HW], FP16, tag="tps")
                nc.tensor.transpose(
                    out=pt[0:32, 0:63], in_=srct[:], identity=identb[0:63, 0:63]
                )
                nc.vector.tensor_copy(out=dst[0:32, off:off + 63], in_=pt[0:32, 0:63])
                nc.sync.dma_start(out=dst[96:128, :], in_=dst[0:32, :])

            qa4 = qaug[0:32, :].rearrange("p (h x y) -> p h x y", h=8, x=32, y=32)
            hdst = qaug[64:96, :].rearrange("p (h x y) -> p h x y", h=8, x=32, y=32)
            wdst = qaug[32:64, :].rearrange("p (h x y) -> p h x y", h=8, x=32, y=32)
            # wnat per 4-head group: cols 4096g + (i32, h4, x32)
            wnp = wnat[32:64, :].rearrange(
                "p (g i h x) -> p g i h x", g=2, i=32, h=4, x=32)

            qr4 = qaug[96:128, :].rearrange("p (h x y) -> p h x y", h=8, x=32, y=32)

            def w_group(g):
                # W rel for heads 4g..4g+3: one MM per y=i (N=128); 8 i per
                # psum tile. Tiles are emitted in PAIRS with the second
                # tile's matmuls on PE row-group 96 (q/krwT replicas), so
                # the two K=32 streams run concurrently. Staged contiguously
                # (i,h,x) on DVE/ACT; the runs-of-1 scatter stays on GPSIMD.
                for half in range(2):
                    tiles = [
                        psR.tile([64, HW], F32, tag="rel", name=f"pw{g}{half}0"),
                        psR.tile([64, HW], F32, tag="rel", name=f"pw{g}{half}1"),
                    ]
                    mms = [t[:].rearrange("p (i h x) -> p i h x", i=8, h=4, x=32)
                           for t in tiles]
                    for ii in range(8):
                        for sub in range(2):
                            i = 8 * (2 * half + sub) + ii
                            if sub == 0:
                                lhsT = krwT[0:32, 31 - i:95 - i]
                                rhs = qa4[:, 4 * g:4 * g + 4, :, i]
                            else:
                                lhsT = krwT[96:128, 31 - i:95 - i]
                                rhs = qr4[:, 4 * g:4 * g + 4, :, i]
                            nc.tensor.matmul(
                                out=mms[sub][:, ii, :, :], lhsT=lhsT, rhs=rhs,
                                start=True, stop=True,
                                tile_position=(96 * sub, 0),
                            )
                    for sub in range(2):
                        ss = 2 * half + sub
                        src = tiles[sub][32:64, :].rearrange(
                            "p (i h x) -> p i h x", i=8, h=4, x=32)
                        dst = wnp[:, g, 8 * ss:8 * ss + 8, :, :]
                        if ss % 2 == 0:
                            nc.scalar.copy(out=dst, in_=src)
                        else:
                            nc.vector.tensor_copy(out=dst, in_=src)

            def w_scatter(h):
                # scatter one head to qaug rows 32-63 (runs of 1) on
                # GPSIMD cores 2-3, which own partitions 32-63.
                nc.gpsimd.tensor_copy(
                    out=wdst[:, h, :, :],
                    in_=wnp[:, h // 4, :, h % 4, :].rearrange("p i x -> p x i"),
                )

            def h_quad(hq, gp):
                # H rel for heads 4hq..4hq+3, x = 16gp..16gp+15 in two
                # row-group-paired psum tiles (row groups 0 / 96 run
                # concurrently); copies go straight to qaug rows 64-95
                # (runs of 32), ScalarE/DVE. Split per head-quad so pair 0
                # only waits on the heads-0-3 half.
                tiles = [
                    psR.tile([96, HW], F32, tag="rel", name=f"ph{hq}{gp}0"),
                    psR.tile([96, HW], F32, tag="rel", name=f"ph{hq}{gp}1"),
                ]
                mms = [t[:].rearrange("p (i h y) -> p i h y", i=8, h=4, y=32)
                       for t in tiles]
                for j in range(8):
                    for sub in range(2):
                        i = 16 * gp + 8 * sub + j
                        if sub == 0:
                            lhsT = krhT[0:32, 31 - i:127 - i]
                            rhs = qa4[:, 4 * hq:4 * hq + 4, i, :]
                        else:
                            lhsT = krhT[96:128, 31 - i:127 - i]
                            rhs = qr4[:, 4 * hq:4 * hq + 4, i, :]
                        nc.tensor.matmul(
                            out=mms[sub][:, j, :, :], lhsT=lhsT, rhs=rhs,
                            start=True, stop=True,
                            tile_position=(96 * sub, 0),
                        )
                for sub in range(2):
                    dst = hdst[:, 4 * hq:4 * hq + 4,
                               16 * gp + 8 * sub:16 * gp + 8 * sub + 8, :]
                    src = tiles[sub][64:96, :].rearrange(
                        "p (i h y) -> p h i y", i=8, h=4, y=32)
                    if sub == 0:
                        nc.scalar.copy(out=dst, in_=src)
                    else:
                        nc.vector.tensor_copy(out=dst, in_=src)

            def v_copy(c):
                nc.gpsimd.tensor_copy(
                    out=v1c[:, c, :, 0:32],
                    in_=x_sb[:, 768 * c + 512:768 * c + 512 + 256]
                        .rearrange("p (h e) -> p h e", h=8),
                )

            # Pair-0-critical work first: W heads 0-3, kT g0, H heads 0-3.
            # Heads-4-7 prep (kT g1, H quad 1, W group 1) trails behind on
            # every queue -- pairs 2-3 don't need it until much later.
            # GPSIMD queue order: scatters h0-h1 (pair 0), v copies (pair-0
            # AV), h2-h3 (pair 1), then the late group's h4-h7.
            w_group(0)
            transpose_group(1, 0, kaug)
            w_scatter(0)
            w_scatter(1)
            h_quad(0, 0)
            h_quad(0, 1)
            for c in range(8):
                v_copy(c)
            w_scatter(2)
            w_scatter(3)
            transpose_group(1, 1, kaug)
            h_quad(1, 0)
            h_quad(1, 1)
            w_group(1)
            for h in range(4, 8):
                w_scatter(h)

        # ================= Phase B: attention per head-pair =================
        with ExitStack() as bctx:
            psS = bctx.enter_context(tc.tile_pool(name="psS", bufs=3, space="PSUM"))
            psT = bctx.enter_context(tc.tile_pool(name="psT", bufs=1, space="PSUM"))
            sbW = bctx.enter_context(tc.tile_pool(name="sbW", bufs=4))
            sbE = bctx.enter_context(tc.tile_pool(name="sbE", bufs=4))
            sbA = bctx.enter_context(tc.tile_pool(name="sbA", bufs=2))
            sbT = bctx.enter_context(tc.tile_pool(name="sbT", bufs=2))
            sbR = bctx.enter_context(tc.tile_pool(name="sbR", bufs=2))

            for hp in range(NH // 2):
                # two heads share one att psum: head 2hp at partitions 0-32,
                # head 2hp+1 at partitions 64-96 (col-tiled concurrent AV).
                # Chunk loop software-pipelined: QK(c)+exp-issue(c), then the
                # pending DVE TT from (c-1), then AV(c-1) -- so the in-order
                # PE never sits behind an exp of its own chunk, and the DVE
                # never idles between TS1 and its TT (GPSIMD shift overlaps).
                att = psT.tile([97, HW], F32, tag="att")
                wexp_prev = None
                pend_tt = []
                for c in range(9):
                    wexps = []
                    if c < 8:
                        for hh in range(2):
                            h = 2 * hp + hh
                            s_ps = psS.tile([128, HW], F32, tag="sT")
                            for e in range(2):
                                nc.tensor.matmul(
                                    out=s_ps[:, 512 * e:512 * e + 512],
                                    lhsT=kaug[:, HW * h + 128 * c:HW * h + 128 * c + 128],
                                    rhs=qaug[0:96, HW * h + 512 * e:HW * h + 512 * e + 512],
                                    start=True, stop=True,
                                )
                            if (hh, c) in DVE_CHUNKS[hp]:
                                e1 = sbE.tile([128, HW], I16, tag="e1")
                                nc.vector.tensor_scalar(
                                    out=e1[:], in0=s_ps[:],
                                    scalar1=SCH_A, scalar2=SCH_B1,
                                    op0=MULT, op1=ADD,
                                )
                                wexps.append(e1[:].bitcast(FP16))
                            else:
                                wexp = sbW.tile([128, HW], FP16, tag="wexp")
                                nc.scalar.activation(
                                    out=wexp[:], in_=s_ps[:], func=EXP, scale=QSCALE,
                                )
                                wexps.append(wexp[:])
                    if c > 0:
                        # AV(c-1), e-major: the two heads' matmuls sit in
                        # different PE col groups and run concurrently.
                        for e in range(2):
                            for hh in range(2):
                                h = 2 * hp + hh
                                nc.tensor.matmul(
                                    out=att[64 * hh:64 * hh + 33, 512 * e:512 * e + 512],
                                    lhsT=v1[:, 264 * h + 33 * (c - 1):264 * h + 33 * (c - 1) + 33],
                                    rhs=wexp_prev[hh][:, 512 * e:512 * e + 512],
                                    start=(c - 1 == 0), stop=(c - 1 == 7),
                                )
                    wexp_prev = wexps

                # Output, pipelined per 512-col (e) half:
                # att -> fp16 SBUF (x 2^-6, cancels in the normalization);
                # ACT takes e=0, DVE takes e=1 so the halves overlap and the
                # (single-buffered) att psum frees for the next pair ASAP.
                att_sb = sbA.tile([112, HW], FP16, tag="attsb")
                nc.gpsimd.memset(att_sb[96:112, :], 0.0)
                nc.scalar.activation(
                    out=att_sb[0:97, 0:512], in_=att[:, 0:512],
                    func=COPY, scale=float(2.0 ** -6),
                )
                nc.vector.tensor_scalar_mul(
                    out=att_sb[0:97, 512:1024],
                    in0=att[:, 512:1024],
                    scalar1=float(2.0 ** -6),
                )
                # xbar DMA transpose per half: [112, 512] -> [128, (c4, j112)]
                att_t = sbT.tile([128, 8 * 112], FP16, tag="attt")
                att_tv = att_t[:].rearrange("p (c j) -> p c j", c=8, j=112)
                rc = sbR.tile([128, 16], F32, tag="rc")
                rcv = rc[:].rearrange("p (c h) -> p c h", c=8, h=2)
                last = hp == NH // 2 - 1
                for e in range(2):
                    # last pair: e=1's transpose issues on the (by then
                    # idle) scalar queue so the two ~1.2us issue slots
                    # overlap instead of serializing on sync.
                    teng = nc.scalar if (last and e == 1) else nc.sync
                    teng.dma_start_transpose(
                        out=att_tv[:, 4 * e:4 * e + 4, :],
                        in_=att_sb[:, 512 * e:512 * e + 512],
                    )
                    # reciprocal of the denominators (row 32 / 96 of att)
                    nc.vector.reciprocal(
                        out=rcv[:, 4 * e:4 * e + 4, :],
                        in_=att_tv[:, 4 * e:4 * e + 4, 32:97:64],
                    )
                    for cc in range(4):
                        cg = 4 * e + cc
                        for hh in range(2):
                            h = 2 * hp + hh
                            dst = out_sb[:, 256 * cg + 32 * h:256 * cg + 32 * h + 32]
                            src = att_tv[:, cg, 64 * hh:64 * hh + 32]
                            sc = rc[:, 2 * cg + hh:2 * cg + hh + 1]
                            # steady state: hh=0 DVE / hh=1 ACT. Last pair
                            # (tail-serial): ~10/6 DVE/ACT split balances
                            # DVE's ~230ns vs ACT's ~400ns per op.
                            if hh == 0 or (last and cc < 2):
                                nc.vector.tensor_scalar_mul(
                                    out=dst, in0=src, scalar1=sc)
                            else:
                                nc.scalar.activation(
                                    out=dst, in_=src, func=COPY, scale=sc)
                    if last:
                        # last pair: half-pixel-range DMA as soon as this
                        # half's scales land (shorter serial tail)
                        nc.sync.dma_start(
                            out=out_d[:].rearrange("(c p) d -> p c d", p=128)
                                [:, 4 * e:4 * e + 4, 64 * hp:64 * hp + 64],
                            in_=out_sb[:].rearrange("p (c d) -> p c d", c=8)
                                [:, 4 * e:4 * e + 4, 64 * hp:64 * hp + 64],
                        )
                if not last:
                    # stream this pair's output columns to DRAM
                    nc.sync.dma_start(
                        out=out_d[:].rearrange("(c p) d -> p c d", p=128)
                            [:, :, 64 * hp:64 * hp + 64],
                        in_=out_sb[:].rearrange("p (c d) -> p c d", c=8)
                            [:, :, 64 * hp:64 * hp + 64],
                    )
    if not nc.is_finalized():
        nc.finalize()
    return nc


_NC = None


def _ensure_axon_hooks_module():
    """bass_utils imports antenv.axon_hooks unconditionally when trace=True;
    this image's antenv lacks it. Provide a stub so tracing degrades to
    no-trace instead of crashing (a real hook can be set by a profiler)."""
    import types

    if "antenv.axon_hooks" in sys.modules:
        return
    try:
        import antenv.axon_hooks  # noqa: F401
        return
    except ImportError:
        pass
    try:
        import antenv
    except ImportError:
        return
    m = types.ModuleType("antenv.axon_hooks")
    m._hook = None
    m.get_axon_ntff_profile_hook = lambda: m._hook
    m.set_axon_ntff_profile_hook = lambda h: setattr(m, "_hook", h)
    sys.modules["antenv.axon_hooks"] = m
    antenv.axon_hooks = m


def kernel(**inputs):
    global _NC
    x = np.ascontiguousarray(np.asarray(inputs["inputs"], dtype=np.float32))
    krw = np.ascontiguousarray(np.asarray(inputs["key_rel_w"], dtype=np.float32))
    krh = np.ascontiguousarray(np.asarray(inputs["key_rel_h"], dtype=np.float32))
    assert x.shape == (8, 32, 32, 768), x.shape
    assert int(inputs["dk"]) == 256 and int(inputs["dv"]) == 256
    assert int(inputs["Nh"]) == 8

    if _NC is None:
        _NC = build_nc()
    _ensure_axon_hooks_module()
    from concourse.bass_utils import run_bass_kernel_spmd

    in_maps = [
        {
            "xa": x[b].reshape(HW, CH)[:HW // 2],
            "xb": x[b].reshape(HW, CH)[HW // 2:],
            "krw": krw,
            "krh": krh,
        }
        for b in range(8)
    ]
    res = run_bass_kernel_spmd(_NC, in_maps, list(range(8)))
    kernel.last_result = res
    out = np.stack([res.results[b]["out"].reshape(32, 32, 256) for b in range(8)], 0)
    return out


if __name__ == "__main__":
    nc = build_nc()
    print("built ok")
